# revision 28
# baseline (speedup 1.0000x reference)
"""AttentionGuidedDynamicRangeDWConv3D on 8 Trainium2 NeuronCores.

Module: out = sum_i softmax(MLP(LN([mean_dhw(x), guidance])))[:, i]
                * dwconv3d(x, convw[i], convb[i], dil=i+1)
Shapes: x [4,96,16,56,56] f32, 3 branches of 3x3x3 depthwise conv with
dilations 1/2/3 ('same' zero padding).

Sharding: 8 cores = (batch b in 0..3) x (channel half hc in 0..1); each
core owns 48 channels of one batch at FULL depth.

Layout trick: partitions = (channel c in 0..8) x (depth d in 0..16), so a
single bf16 matmul with a 128x128 block-banded weight matrix applies an
entire depth-band of conv taps at once: out[(c,d), hw] +=
sum_od w[c, (od,oh,ow)] * x[(c,d+od), hw + oh*56+ow].  The 81 taps
(3 branches x 27) collapse into 25 matmul passes -- one per distinct
(oh,ow) pair -- accumulated in PSUM per 448-column (8 h-row) chunk.
Depth 'same' padding falls out of band truncation (no halo).  H/W 'same'
padding is exact via trimmed 2D access patterns (bf16 matmuls allow
strided APs; fp32r would not).

Band matrices are built by the Vector engine from host-supplied
shifted-identity masks scaled by per-partition weight columns.  The gate
MLP runs redundantly per core; the global mean-pool takes one pairwise
128x6-float AllGather of raw plane sums (15us fixed latency in the cost
model).  To hide that latency plus the MLP chain, slabs 0 and 1 run
UNGATED: their 27 single-branch passes accumulate the three branch
convs into separate PSUM banks per chunk, the Scalar engine copies them
to SBUF, and once the softmax weights land the Vector engine does the
weighted 3-way merge.  Slabs 2-5 use gate-folded matrices (25 passes)
and a plain Act PSUM->SBUF copy -- except slab 5, where 4 branch-1
passes (12 taps) run on the otherwise-idle Vector engine instead of the
PE: depth-shifted taps read partition-rotated SBUF copies of the slab
(rows crossing a channel block are neutralized by zeroing their weight
via a host depth mask), accumulate into an SBUF buffer zeroed by
GPSIMD, and fold into the store path with one add per chunk.  Engines
execute in order, so emission order keeps DVE (matrix builds + assist),
Act (pool reductions + PSUM copies) and the collective chain ahead of
the Tensor engine throughout; a few dummy matmuls on the mask tile warm
the PE p-state while the first x slab is still loading.
"""

import sys

if "/opt/trn_rl_repo" not in sys.path:
    sys.path.insert(0, "/opt/trn_rl_repo")

import ml_dtypes
import numpy as np

import concourse.bass as bass
import concourse.mybir as mybir
import concourse.tile as tile
from concourse.bass_utils import run_bass_kernel_spmd

F32 = mybir.dt.float32
BF16 = mybir.dt.bfloat16
ALU = mybir.AluOpType
ACTF = mybir.ActivationFunctionType

B, C, D, H, W = 4, 96, 16, 56, 56
G, HID, NB = 96, 24, 3
K = 3
DILS = (1, 2, 3)
LN_EPS = 1e-5
N_CORES = 8
NCH = 8                  # channels per slab
NSLAB = 6                # slabs per core (48 channels)
NUNG = 2                 # ungated slabs (hide the collective+MLP latency)
NP = NCH * D             # 128 partitions
HW = H * W               # 3136
CHUNK = 448              # 8 h-rows; PSUM bank-sized chunk
N_CHUNKS = 7
ROWS = CHUNK // W        # 8


def _pass_list(split_branches):
    """[(oh, ow, [(od, t), ...])].  split_branches: one pass per (branch,
    (oh,ow)) with the branch's (0,0) pass first (27 passes); else one pass
    per distinct (oh,ow) with (0,0) merged across branches first (25)."""
    out = []
    for i, d in enumerate(DILS):
        for oh in (0, -d, d):
            for ow in (0, -d, d) if oh == 0 else (-d, 0, d):
                if split_branches:
                    kh, kw = oh // d + 1, ow // d + 1
                    ents = [(kd * d - d, i * 27 + kd * 9 + kh * 3 + kw)
                            for kd in range(K)]
                    out.append((oh, ow, ents))
                else:
                    if (oh, ow) == (0, 0) and i > 0:
                        continue
                    ents = []
                    for j, dj in enumerate(DILS):
                        if oh in (-dj, 0, dj) and ow in (-dj, 0, dj):
                            kh, kw = oh // dj + 1, ow // dj + 1
                            ents += [(kd * dj - dj,
                                      j * 27 + kd * 9 + kh * 3 + kw)
                                     for kd in range(K)]
                    out.append((oh, ow, ents))
    if split_branches:
        assert len(out) == 27
    else:
        assert len(out) == 25
    assert sum(len(e) for _, _, e in out) == 81
    return out


def _build_program(with_bias):
    nc = bass.Bass()
    xin = nc.dram_tensor("x", [NP, NSLAB * HW], BF16, kind="ExternalInput")
    masks_in = nc.dram_tensor("masks", [NP, 7 * NP], BF16, kind="ExternalInput")
    cwx_in = nc.dram_tensor("cwx", [NP, NSLAB * 27 * NB], F32, kind="ExternalInput")
    cbx_in = nc.dram_tensor("cbx", [NP, NSLAB * NB], F32, kind="ExternalInput")
    gdin = nc.dram_tensor("gd", [G], F32, kind="ExternalInput")
    w1t_in = nc.dram_tensor("w1t", [HID, C + G], F32, kind="ExternalInput")
    b1_in = nc.dram_tensor("b1", [HID], F32, kind="ExternalInput")
    w2_in = nc.dram_tensor("w2", [HID, NB], F32, kind="ExternalInput")
    b2_in = nc.dram_tensor("b2", [NB], F32, kind="ExternalInput")
    lng_in = nc.dram_tensor("lng", [C + G], F32, kind="ExternalInput")
    lnb_in = nc.dram_tensor("lnb", [C + G], F32, kind="ExternalInput")
    dm_in = nc.dram_tensor("dm", [NP, 2], F32, kind="ExternalInput")
    yout = nc.dram_tensor("y", [NP, NSLAB * HW], F32, kind="ExternalOutput")

    p_ung = _pass_list(True)
    p_gat = _pass_list(False)

    with tile.TileContext(nc) as tc:
        with (
            tc.tile_pool(name="sbuf", bufs=1) as pool,
            tc.tile_pool(name="mats", bufs=2) as matpool,
            tc.tile_pool(name="outs", bufs=4) as outpool,
            tc.tile_pool(name="dram", bufs=1, space="DRAM") as dpool,
            tc.tile_pool(name="psum", bufs=1, space="PSUM") as ppool,
        ):
            xs = [
                pool.tile([NP, HW], BF16, tag=f"xs{s}", name=f"xs{s}")
                for s in range(NSLAB)
            ]
            masks = pool.tile([NP, 7 * NP], BF16, tag="masks")
            cwx = pool.tile([NP, NSLAB * 27 * NB], F32, tag="cwx")
            w_exp = pool.tile([NP, NSLAB * 27 * NB], F32, tag="w_exp")
            scr = pool.tile([NP, HW], BF16, tag="scr")
            part = pool.tile([NP, NSLAB], F32, tag="part")
            grow = pool.tile([1, 2 * NP * NSLAB], F32, tag="grow")
            xr_p1 = pool.tile([NP, HW], BF16, tag="xr_p1")
            xr_m1 = pool.tile([NP, HW], BF16, tag="xr_m1")
            acc5 = pool.tile([NP, HW], F32, tag="acc5")
            wz = pool.tile([NP, 12], F32, tag="wz")
            dmask = pool.tile([NP, 2], F32, tag="dmask")
            bb = [
                [
                    pool.tile([NP, HW], F32, tag=f"bb{s}_{b}",
                              name=f"bb{s}_{b}")
                    for b in range(NB)
                ]
                for s in range(NUNG)
            ]
            g_row = pool.tile([1, C + G], F32, tag="g_row")
            gd_row = pool.tile([1, C + G], F32, tag="gd_row")
            lng = pool.tile([1, C + G], F32, tag="lng")
            lnb = pool.tile([1, C + G], F32, tag="lnb")
            gn_row = pool.tile([1, C + G], F32, tag="gn_row")
            gn_bc = pool.tile([HID, C + G], F32, tag="gn_bc")
            w1t = pool.tile([HID, C + G], F32, tag="w1t")
            prod = pool.tile([HID, C + G], F32, tag="prod")
            hvec = pool.tile([HID, 1], F32, tag="hvec")
            b1c = pool.tile([HID, 1], F32, tag="b1c")
            w2t = pool.tile([HID, NB], F32, tag="w2t")
            l2tmp = pool.tile([HID, NB], F32, tag="l2tmp")
            z72 = pool.tile([1, HID * NB], F32, tag="z72")
            zrow = pool.tile([1, NB], F32, tag="zrow")
            b2r = pool.tile([1, NB], F32, tag="b2r")
            wts = pool.tile([1, NB], F32, tag="wts")
            wts_bc = pool.tile([NP, NB], F32, tag="wts_bc")
            s1 = pool.tile([1, 1], F32, tag="s1")
            s2 = pool.tile([1, 1], F32, tag="s2")
            s3 = pool.tile([1, 1], F32, tag="s3")
            s4 = pool.tile([1, 1], F32, tag="s4")
            if with_bias:
                cbx = pool.tile([NP, NSLAB * NB], F32, tag="cbx")
                b_exp = pool.tile([NP, NSLAB], F32, tag="b_exp")
                betmp = pool.tile([NP, NSLAB * NB], F32, tag="betmp")

            cin = dpool.tile([NP, NSLAB], F32, tag="cin")
            cout = dpool.tile([2 * NP, NSLAB], F32, tag="cout")
            gb = dpool.tile([1, C + G], F32, tag="gb")
            wb = dpool.tile([1, NB], F32, tag="wb")

            v = nc.vector
            sc = nc.scalar

            # ---- A: loads (small weights first, then x slab-by-slab) ----
            nc.sync.dma_start(out=masks[:, :], in_=masks_in[:, :])
            nc.sync.dma_start(out=cwx[:, :], in_=cwx_in[:, :])
            nc.sync.dma_start(out=xs[0][:, :], in_=xin[:, 0:HW])
            for s in range(1, NSLAB):
                nc.sync.dma_start(
                    out=xs[s][:, :], in_=xin[:, s * HW : (s + 1) * HW]
                )
            nc.sync.dma_start(out=w1t[:, :], in_=w1t_in[:, :])
            nc.sync.dma_start(out=b1c[:, :], in_=b1_in[:, None])
            nc.sync.dma_start(out=w2t[:, :], in_=w2_in[:, :])
            nc.sync.dma_start(out=b2r[:, :], in_=b2_in[None, :])
            nc.sync.dma_start(out=lng[:, :], in_=lng_in[None, :])
            nc.sync.dma_start(out=lnb[:, :], in_=lnb_in[None, :])
            nc.sync.dma_start(out=g_row[:, C:], in_=gdin[None, :])
            nc.sync.dma_start(out=dmask[:, :], in_=dm_in[:, :])
            if with_bias:
                nc.sync.dma_start(out=cbx[:, :], in_=cbx_in[:, :])

            # ---- B: plane sums: slabs 0-3 on Act now; slab 4 threaded
            # between slab-0 branch copies; slab 5 on DVE (Act stays just
            # ahead of both the PSUM-copy demand and the collective) ----
            for s in range(NSLAB - 2):
                sc.activation(
                    scr[:, :], xs[s][:, :], ACTF.Copy,
                    accum_out=part[:, s : s + 1],
                )

            # ---- C: pairwise AllGather of raw plane sums ----
            nc.sync.dma_start(out=cin[:, :], in_=part[:, :])
            nc.gpsimd.collective_compute(
                "AllGather",
                ALU.bypass,
                replica_groups=[[2 * b, 2 * b + 1] for b in range(B)],
                ins=[cin.opt()],
                outs=[cout.opt()],
            )
            nc.sync.dma_start(out=grow[:, :], in_=cout[:, :])

            # ---- conv helpers ----
            def build_mats(s, passes, wsrc):
                mats = []
                for mi, (oh, ow, entries) in enumerate(passes):
                    mt = matpool.tile([NP, NP], BF16, tag=f"m{mi}")
                    for ei, (od, t) in enumerate(entries):
                        mk_in = masks[:, (od + 3) * NP : (od + 4) * NP]
                        wcol = wsrc[:, s * 81 + t : s * 81 + t + 1]
                        if ei == 0:
                            v.tensor_scalar(
                                out=mt[:, :], in0=mk_in, scalar1=wcol,
                                scalar2=None, op0=ALU.mult,
                            )
                        else:
                            v.scalar_tensor_tensor(
                                out=mt[:, :], in0=mk_in, scalar=wcol,
                                in1=mt[:, :], op0=ALU.mult, op1=ALU.add,
                            )
                    mats.append(mt)
                return mats

            def emit_pass(ps, pv, mt, xf, xv, ci, oh, ow, start, stop):
                if (oh, ow) == (0, 0):
                    nc.tensor.matmul(
                        ps[:, :], mt[:, :],
                        xf[:, ci * CHUNK : (ci + 1) * CHUNK],
                        start=start, stop=stop, skip_group_check=True,
                    )
                    return
                h0 = max(ci * ROWS, -oh if oh < 0 else 0)
                h1 = min(ci * ROWS + ROWS, H - (oh if oh > 0 else 0))
                if h1 <= h0:
                    return
                w0 = -ow if ow < 0 else 0
                w1 = W - (ow if ow > 0 else 0)
                nc.tensor.matmul(
                    pv[:, h0 - ci * ROWS : h1 - ci * ROWS, w0:w1],
                    mt[:, :],
                    xv[:, h0 + oh : h1 + oh, w0 + ow : w1 + ow],
                    start=start, stop=stop, skip_group_check=True,
                )

            def slab_views(s):
                xf = xs[s][:, :]
                return xf, xf.rearrange("p (h w) -> p h w", h=H, w=W)

            def emit_ungated_chunkmajor(s, mats):
                xf, xv = slab_views(s)
                for ci in range(N_CHUNKS):
                    for b in range(NB):
                        ps = ppool.tile([NP, CHUNK], F32,
                                        tag=f"ps{(3 * ci + b) % 8}",
                                        name=f"ups{s}_{b}_{ci}")
                        pv = ps[:, :].rearrange("p (h w) -> p h w", h=ROWS, w=W)
                        for k in range(9):
                            oh, ow, _ = p_ung[b * 9 + k]
                            emit_pass(ps, pv, mats[b * 9 + k], xf, xv, ci,
                                      oh, ow, k == 0, k == 8)
                        sc.activation(
                            bb[s][b][:, ci * CHUNK : (ci + 1) * CHUNK],
                            ps[:, :], ACTF.Copy,
                        )

            def emit_merge(s):
                for ci in range(N_CHUNKS):
                    sl = slice(ci * CHUNK, (ci + 1) * CHUNK)
                    ot = outpool.tile([NP, CHUNK], F32, tag=f"o{ci % 4}")
                    v.tensor_scalar(
                        out=ot[:, :], in0=bb[s][0][:, sl],
                        scalar1=wts_bc[:, 0:1], scalar2=None, op0=ALU.mult,
                    )
                    for b in (1, 2):
                        v.scalar_tensor_tensor(
                            out=ot[:, :], in0=bb[s][b][:, sl],
                            scalar=wts_bc[:, b : b + 1], in1=ot[:, :],
                            op0=ALU.mult, op1=ALU.add,
                        )
                    if with_bias:
                        v.tensor_scalar(
                            out=ot[:, :], in0=ot[:, :],
                            scalar1=b_exp[:, s : s + 1], scalar2=None,
                            op0=ALU.add,
                        )
                    nc.sync.dma_start(
                        out=yout[:, s * HW + ci * CHUNK : s * HW + (ci + 1) * CHUNK],
                        in_=ot[:, :],
                    )

            def emit_gated_out(s, ci, ps, acc=None):
                ot = outpool.tile([NP, CHUNK], F32, tag=f"o{ci % 4}",
                                  name=f"ot{s}_{ci}")
                sc.activation(ot[:, :], ps[:, :], ACTF.Copy)
                if acc is not None:
                    v.tensor_tensor(
                        out=ot[:, :], in0=ot[:, :],
                        in1=acc[:, ci * CHUNK : (ci + 1) * CHUNK], op=ALU.add,
                    )
                if with_bias:
                    v.tensor_scalar(
                        out=ot[:, :], in0=ot[:, :],
                        scalar1=b_exp[:, s : s + 1], scalar2=None,
                        op0=ALU.add,
                    )
                nc.sync.dma_start(
                    out=yout[:, s * HW + ci * CHUNK : s * HW + (ci + 1) * CHUNK],
                    in_=ot[:, :],
                )

            def emit_gated_slab(s, passes=None, acc=None):
                passes = passes or p_gat
                mats = build_mats(s, passes, w_exp)
                if acc is not None:
                    emit_assist()
                xf, xv = slab_views(s)
                for ci in range(N_CHUNKS):
                    ps = ppool.tile([NP, CHUNK], F32, tag=f"ps{ci}",
                                    name=f"gps{s}_{ci}")
                    pv = ps[:, :].rearrange("p (h w) -> p h w", h=ROWS, w=W)
                    for mi, (oh, ow, _) in enumerate(passes):
                        emit_pass(ps, pv, mats[mi], xf, xv, ci, oh, ow,
                                  mi == 0, mi == len(passes) - 1)
                    emit_gated_out(s, ci, ps, acc)

            # ---- D: slab 0 ungated (builds + matmuls + copies) ----
            def red4():
                sc.activation(
                    scr[:, :], xs[4][:, :], ACTF.Copy,
                    accum_out=part[:, 4:5],
                )

            # p-state warmup: dummy matmuls on the masks tile while the
            # xs0 DMA is still in flight, so the real conv stream starts at
            # full clock (the tensor engine needs ~3us of continuous work)
            wps = ppool.tile([NP, CHUNK], F32, tag="ps7", name="warmps")
            for wi in range(8):
                nc.tensor.matmul(
                    wps[:, :], masks[:, 3 * NP : 4 * NP],
                    masks[:, 2 * NP : 2 * NP + CHUNK],
                    start=(wi == 0), stop=(wi == 7), skip_group_check=True,
                )

            mats0 = build_mats(0, p_ung, cwx)
            emit_ungated_chunkmajor(0, mats0)
            red4()

            # slab-5 plane sum on DVE (Act is busy; DVE has a lull here)
            v.reduce_sum(
                part[:, NSLAB - 1 : NSLAB], xs[NSLAB - 1][:, :],
                axis=mybir.AxisListType.X,
            )

            # ---- F1: slab 1 ungated ----
            mats1 = build_mats(1, p_ung, cwx)
            emit_ungated_chunkmajor(1, mats1)

            # ---- E: gate MLP ----
            # feat[48r + 8s + c] = sum_d cout[r, (c,d), s] / (D*HW)
            for r in range(2):
                gview = grow[:, r * NP * NSLAB : (r + 1) * NP * NSLAB].rearrange(
                    "a (c d s) -> a s c d", c=NCH, d=D, s=NSLAB
                )
                tview = g_row[:, 48 * r : 48 * r + 48].rearrange(
                    "a (s c) -> a s c", s=NSLAB, c=NCH
                )
                v.reduce_sum(tview, gview, axis=mybir.AxisListType.X)
            v.tensor_scalar_mul(g_row[:, 0:C], g_row[:, 0:C], 1.0 / (D * HW))

            # LayerNorm over 192 on one partition
            v.reduce_sum(s1[:, :], g_row[:, :], axis=mybir.AxisListType.X)
            v.tensor_scalar_mul(s1[:, :], s1[:, :], 1.0 / (C + G))  # mu
            v.tensor_scalar(
                out=gd_row[:, :], in0=g_row[:, :], scalar1=s1[:, :], scalar2=None,
                op0=ALU.subtract,
            )
            v.tensor_tensor(out=gn_row[:, :], in0=gd_row[:, :], in1=gd_row[:, :], op=ALU.mult)
            v.reduce_sum(s2[:, :], gn_row[:, :], axis=mybir.AxisListType.X)
            v.tensor_scalar(
                out=s2[:, :], in0=s2[:, :], scalar1=1.0 / (C + G), scalar2=LN_EPS,
                op0=ALU.mult, op1=ALU.add,
            )  # var + eps
            sc.activation(s3[:, :], s2[:, :], ACTF.Sqrt)
            # one Newton step for a clean sqrt
            v.reciprocal(s4[:, :], s3[:, :])
            v.tensor_tensor(out=s4[:, :], in0=s4[:, :], in1=s2[:, :], op=ALU.mult)
            v.tensor_tensor(out=s4[:, :], in0=s4[:, :], in1=s3[:, :], op=ALU.add)
            v.tensor_scalar_mul(s4[:, :], s4[:, :], 0.5)
            v.reciprocal(s3[:, :], s4[:, :])  # rstd
            v.tensor_scalar(
                out=gn_row[:, :], in0=gd_row[:, :], scalar1=s3[:, :], scalar2=None,
                op0=ALU.mult,
            )
            v.tensor_tensor(out=gn_row[:, :], in0=gn_row[:, :], in1=lng[:, :], op=ALU.mult)
            v.tensor_tensor(out=gn_row[:, :], in0=gn_row[:, :], in1=lnb[:, :], op=ALU.add)

            # MLP layer 1: h = gelu(gn @ w1 + b1) via row-products
            nc.sync.dma_start(out=gb[:, :], in_=gn_row[:, :])
            nc.sync.dma_start(out=gn_bc[:, :], in_=gb[:1, :].partition_broadcast(HID))
            v.tensor_tensor(out=prod[:, :], in0=w1t[:, :], in1=gn_bc[:, :], op=ALU.mult)
            v.reduce_sum(hvec[:, :], prod[:, :], axis=mybir.AxisListType.X)
            v.tensor_tensor(out=hvec[:, :], in0=hvec[:, :], in1=b1c[:, :], op=ALU.add)
            sc.activation(hvec[:, :], hvec[:, :], ACTF.Gelu)

            # MLP layer 2 via DRAM transpose bounce
            v.tensor_scalar(
                out=l2tmp[:, :], in0=w2t[:, :], scalar1=hvec[:, :], scalar2=None,
                op0=ALU.mult,
            )
            nc.sync.dma_start(out=z72[:, :], in_=l2tmp[:, :])
            z3 = z72[:, :].rearrange("a (j i) -> a j i", j=HID, i=NB)
            for i in range(NB):
                v.reduce_sum(zrow[:, i : i + 1], z3[:, :, i], axis=mybir.AxisListType.X)
            v.tensor_tensor(out=zrow[:, :], in0=zrow[:, :], in1=b2r[:, :], op=ALU.add)

            # softmax over 3
            v.reduce_max(s1[:, :], zrow[:, :], axis=mybir.AxisListType.X)
            v.tensor_scalar(
                out=zrow[:, :], in0=zrow[:, :], scalar1=s1[:, :], scalar2=None,
                op0=ALU.subtract,
            )
            sc.activation(zrow[:, :], zrow[:, :], ACTF.Exp)
            v.reduce_sum(s2[:, :], zrow[:, :], axis=mybir.AxisListType.X)
            v.reciprocal(s2[:, :], s2[:, :])
            v.tensor_scalar(
                out=wts[:, :], in0=zrow[:, :], scalar1=s2[:, :], scalar2=None,
                op0=ALU.mult,
            )

            # broadcast gate weights; fold into per-channel tap weights
            nc.sync.dma_start(out=wb[:, :], in_=wts[:, :])
            nc.sync.dma_start(out=wts_bc[:, :], in_=wb[:1, :].partition_broadcast(NP))
            for s in range(NUNG, NSLAB):
                for i in range(NB):
                    sl = slice(s * 81 + i * 27, s * 81 + (i + 1) * 27)
                    v.tensor_scalar(
                        out=w_exp[:, sl], in0=cwx[:, sl],
                        scalar1=wts_bc[:, i : i + 1], scalar2=None, op0=ALU.mult,
                    )
            if with_bias:
                for i in range(NB):
                    v.tensor_scalar(
                        out=betmp[:, i::NB], in0=cbx[:, i::NB],
                        scalar1=wts_bc[:, i : i + 1], scalar2=None, op0=ALU.mult,
                    )
                v.tensor_tensor(
                    out=b_exp[:, :], in0=betmp[:, 0::NB], in1=betmp[:, 1::NB],
                    op=ALU.add,
                )
                v.tensor_tensor(
                    out=b_exp[:, :], in0=b_exp[:, :], in1=betmp[:, 2::NB],
                    op=ALU.add,
                )

            # slab 5: move 4 branch-1 passes off the PE onto the Vector
            # engine, which is idle for the last ~70us.  od=+-1 depth taps
            # read partition-rotated copies of the slab; rows whose rotated
            # partner crosses a channel block get weight 0 via dmask.
            ASSIST = [(0, -1), (0, 1), (-1, 0), (1, 0)]
            nc.sync.dma_start(out=xr_p1[0 : NP - 1, :], in_=xs[5][1:NP, :])
            nc.sync.dma_start(out=xr_p1[NP - 1 : NP, :], in_=xs[5][NP - 1 : NP, :])
            nc.sync.dma_start(out=xr_m1[1:NP, :], in_=xs[5][0 : NP - 1, :])
            nc.sync.dma_start(out=xr_m1[0:1, :], in_=xs[5][0:1, :])
            nc.gpsimd.memset(acc5[:, :], 0.0)

            def emit_assist():
                k = 0
                accv = acc5[:, :].rearrange("p (h w) -> p h w", h=H, w=W)
                for oh, ow in ASSIST:
                    for kd in range(K):
                        od = kd - 1
                        t = kd * 9 + (oh + 1) * 3 + (ow + 1)
                        wcol = w_exp[:, 5 * 81 + t : 5 * 81 + t + 1]
                        if od == 0:
                            src, wc = xs[5], wcol
                        else:
                            mi = 0 if od == 1 else 1
                            v.tensor_tensor(
                                out=wz[:, k : k + 1], in0=wcol,
                                in1=dmask[:, mi : mi + 1], op=ALU.mult,
                            )
                            src = xr_p1 if od == 1 else xr_m1
                            wc = wz[:, k : k + 1]
                            k += 1
                        sv = src[:, :].rearrange("p (h w) -> p h w", h=H, w=W)
                        h0, h1 = max(0, -oh), H - max(0, oh)
                        w0, w1 = max(0, -ow), W - max(0, ow)
                        v.scalar_tensor_tensor(
                            out=accv[:, h0:h1, w0:w1],
                            in0=sv[:, h0 + oh : h1 + oh, w0 + ow : w1 + ow],
                            scalar=wc, in1=accv[:, h0:h1, w0:w1],
                            op0=ALU.mult, op1=ALU.add,
                        )

            p_lite = [e for e in p_gat if (e[0], e[1]) not in ASSIST]

            # ---- G..J: gated slabs; merges (not PE-critical) trail ----
            emit_gated_slab(2)
            emit_merge(0)
            emit_gated_slab(3)
            emit_merge(1)
            emit_gated_slab(4)
            emit_gated_slab(5, passes=p_lite, acc=acc5)

    _split_sem_waits(nc)
    return nc


_WAITSPLIT = [0]


def _split_sem_waits(nc, max_waits=1):
    """This walrus build rejects >1 SyncWait per instruction (and any wait on
    a Drain). Move excess waits onto same-engine NOPs inserted just before."""
    for bb in nc.main_func.blocks:
        insns = bb.instructions
        i = 0
        while i < len(insns):
            ins = insns[i]
            si = ins.sync_info
            limit = 0 if ins.opcode == "Drain" else max_waits
            if si is not None and si.on_wait is not None and len(si.on_wait) > limit:
                waits = list(si.on_wait)
                keep = waits[-limit:] if limit else []
                extra = waits[: len(waits) - limit]
                pos = i
                for j in range(0, len(extra), max_waits):
                    nop = mybir.InstNoOp(
                        name=f"I-waitsplit-{_WAITSPLIT[0]}", ins=[], outs=[]
                    )
                    _WAITSPLIT[0] += 1
                    nop.engine = ins.engine
                    nop.sync_info = mybir.SyncInfo(
                        on_wait=extra[j : j + max_waits], on_update=[]
                    )
                    insns.insert(pos, nop)
                    pos += 1
                    i += 1
                si.on_wait = keep
            i += 1


def _make_masks():
    m = np.zeros((NP, 7 * NP), dtype=np.float32)
    for od in range(-3, 4):
        for p in range(NP):
            q = p - od
            if q // D == p // D and 0 <= q < NP:
                m[p, (od + 3) * NP + q] = 1.0
    return m.astype(ml_dtypes.bfloat16)


def _prep_inputs(x, guidance, convw, convb, ln_g, ln_b, w1, b1, w2, b2):
    f = np.float32
    w3 = np.ascontiguousarray(convw.reshape(NB, C, 27), dtype=f)
    cb = np.ascontiguousarray(convb, dtype=f)
    dm = np.ones((NP, 2), dtype=np.float32)
    dm[D - 1 :: D, 0] = 0.0   # od=+1: d==15 rows read the next c-block
    dm[0::D, 1] = 0.0         # od=-1: d==0 rows read the previous c-block
    common = dict(
        masks=_make_masks(),
        dm=dm,
        w1t=np.ascontiguousarray(w1.T, dtype=f),
        b1=np.ascontiguousarray(b1, dtype=f),
        w2=np.ascontiguousarray(w2, dtype=f),
        b2=np.ascontiguousarray(b2, dtype=f),
        lng=np.ascontiguousarray(ln_g, dtype=f),
        lnb=np.ascontiguousarray(ln_b, dtype=f),
    )
    in_maps = []
    for core in range(N_CORES):
        b, hc = core // 2, core % 2
        ch0 = 48 * hc
        # xs[p=c*16+d, s*HW+j] = x[b, ch0+8s+c, d, j]
        arr = np.ascontiguousarray(x[b, ch0 : ch0 + 48], dtype=f)
        arr = arr.reshape(NSLAB, NCH, D, HW).transpose(1, 2, 0, 3).reshape(
            NP, NSLAB * HW
        )
        # cwx[p=c*16+d, s*81+t] = convw[br, ch0+8s+c, t27]  (d-independent)
        cw = w3[:, ch0 : ch0 + 48, :].reshape(NB, NSLAB, NCH, 27)
        cw = cw.transpose(2, 1, 0, 3).reshape(NCH, NSLAB * NB * 27)
        cwx = np.repeat(cw, D, axis=0)  # row c*16+d <- cw[c]
        cbs = cb[:, ch0 : ch0 + 48].reshape(NB, NSLAB, NCH)
        cbs = cbs.transpose(2, 1, 0).reshape(NCH, NSLAB * NB)
        cbx = np.repeat(cbs, D, axis=0)
        in_maps.append(
            dict(
                x=arr.astype(ml_dtypes.bfloat16),
                gd=np.ascontiguousarray(guidance[b], dtype=f),
                cwx=np.ascontiguousarray(cwx, dtype=f),
                cbx=np.ascontiguousarray(cbx, dtype=f),
                **common,
            )
        )
    return in_maps


_CACHED_NC = {}


def kernel(x, guidance, convw, convb, ln_g, ln_b, w1, b1, w2, b2):
    with_bias = bool(np.any(np.asarray(convb)))
    if with_bias not in _CACHED_NC:
        _CACHED_NC[with_bias] = _build_program(with_bias)
    nc = _CACHED_NC[with_bias]
    globals()["_LAST_NC"] = nc
    in_maps = _prep_inputs(
        x, guidance, convw, convb, ln_g, ln_b, w1, b1, w2, b2
    )
    res = run_bass_kernel_spmd(nc, in_maps, list(range(N_CORES)))
    out = np.empty((B, C, D, H, W), dtype=np.float32)
    for core in range(N_CORES):
        b, hc = core // 2, core % 2
        y = res.results[core]["y"].reshape(NCH, D, NSLAB, HW)
        out[b, 48 * hc : 48 * hc + 48] = (
            y.transpose(2, 0, 1, 3).reshape(48, D, H, W)
        )
    return out


if __name__ == "__main__":
    rng = np.random.default_rng(0)
    ins = dict(
        x=rng.standard_normal((B, C, D, H, W), dtype=np.float32),
        guidance=rng.standard_normal((B, G), dtype=np.float32),
        convw=(rng.standard_normal((NB, C, 1, K, K, K)) * 0.1).astype(np.float32),
        convb=np.zeros((NB, C), np.float32),
        ln_g=np.ones((C + G,), np.float32),
        ln_b=np.zeros((C + G,), np.float32),
        w1=(rng.standard_normal((C + G, HID)) * 0.05).astype(np.float32),
        b1=np.zeros((HID,), np.float32),
        w2=(rng.standard_normal((HID, NB)) * 0.05).astype(np.float32),
        b2=np.zeros((NB,), np.float32),
    )
    out = kernel(**ins)
    print("kernel ran, out shape", out.shape, "mean", float(np.abs(out).mean()))


# revision 30
# speedup vs baseline: 1.0025x; 1.0025x over previous
"""AttentionGuidedDynamicRangeDWConv3D on 8 Trainium2 NeuronCores.

Module: out = sum_i softmax(MLP(LN([mean_dhw(x), guidance])))[:, i]
                * dwconv3d(x, convw[i], convb[i], dil=i+1)
Shapes: x [4,96,16,56,56] f32, 3 branches of 3x3x3 depthwise conv with
dilations 1/2/3 ('same' zero padding).

Sharding: 8 cores = (batch b in 0..3) x (channel half hc in 0..1); each
core owns 48 channels of one batch at FULL depth.

Layout trick: partitions = (channel c in 0..8) x (depth d in 0..16), so a
single bf16 matmul with a 128x128 block-banded weight matrix applies an
entire depth-band of conv taps at once: out[(c,d), hw] +=
sum_od w[c, (od,oh,ow)] * x[(c,d+od), hw + oh*56+ow].  The 81 taps
(3 branches x 27) collapse into 25 matmul passes -- one per distinct
(oh,ow) pair -- accumulated in PSUM per 448-column (8 h-row) chunk.
Depth 'same' padding falls out of band truncation (no halo).  H/W 'same'
padding is exact via trimmed 2D access patterns (bf16 matmuls allow
strided APs; fp32r would not).

Band matrices are built by the Vector engine from host-supplied
shifted-identity masks scaled by per-partition weight columns.  The gate
MLP runs redundantly per core; the global mean-pool takes one pairwise
128x6-float AllGather of raw plane sums (15us fixed latency in the cost
model).  To hide that latency plus the MLP chain, slabs 0 and 1 run
UNGATED: their 27 single-branch passes accumulate the three branch
convs into separate PSUM banks per chunk, the Scalar engine copies them
to SBUF, and once the softmax weights land the Vector engine does the
weighted 3-way merge.  Slabs 2-5 use gate-folded matrices (25 passes)
and a plain Act PSUM->SBUF copy -- except slab 5, where 4 branch-1
passes (12 taps) run on the otherwise-idle Vector engine instead of the
PE: depth-shifted taps read partition-rotated SBUF copies of the slab
(rows crossing a channel block are neutralized by zeroing their weight
via a host depth mask), accumulate into an SBUF buffer zeroed by
GPSIMD, and fold into the store path with one add per chunk.  Engines
execute in order, so emission order keeps DVE (matrix builds + assist),
Act (pool reductions + PSUM copies) and the collective chain ahead of
the Tensor engine throughout; a few dummy matmuls on the mask tile warm
the PE p-state while the first x slab is still loading.
"""

import sys

if "/opt/trn_rl_repo" not in sys.path:
    sys.path.insert(0, "/opt/trn_rl_repo")

import ml_dtypes
import numpy as np

import concourse.bass as bass
import concourse.mybir as mybir
import concourse.tile as tile
from concourse.bass_utils import run_bass_kernel_spmd

F32 = mybir.dt.float32
BF16 = mybir.dt.bfloat16
ALU = mybir.AluOpType
ACTF = mybir.ActivationFunctionType

B, C, D, H, W = 4, 96, 16, 56, 56
G, HID, NB = 96, 24, 3
K = 3
DILS = (1, 2, 3)
LN_EPS = 1e-5
N_CORES = 8
NCH = 8                  # channels per slab
NSLAB = 6                # slabs per core (48 channels)
NUNG = 2                 # ungated slabs (hide the collective+MLP latency)
NP = NCH * D             # 128 partitions
HW = H * W               # 3136
CHUNK = 448              # 8 h-rows; PSUM bank-sized chunk
N_CHUNKS = 7
ROWS = CHUNK // W        # 8


def _pass_list(split_branches):
    """[(oh, ow, [(od, t), ...])].  split_branches: one pass per (branch,
    (oh,ow)) with the branch's (0,0) pass first (27 passes); else one pass
    per distinct (oh,ow) with (0,0) merged across branches first (25)."""
    out = []
    for i, d in enumerate(DILS):
        for oh in (0, -d, d):
            for ow in (0, -d, d) if oh == 0 else (-d, 0, d):
                if split_branches:
                    kh, kw = oh // d + 1, ow // d + 1
                    ents = [(kd * d - d, i * 27 + kd * 9 + kh * 3 + kw)
                            for kd in range(K)]
                    out.append((oh, ow, ents))
                else:
                    if (oh, ow) == (0, 0) and i > 0:
                        continue
                    ents = []
                    for j, dj in enumerate(DILS):
                        if oh in (-dj, 0, dj) and ow in (-dj, 0, dj):
                            kh, kw = oh // dj + 1, ow // dj + 1
                            ents += [(kd * dj - dj,
                                      j * 27 + kd * 9 + kh * 3 + kw)
                                     for kd in range(K)]
                    out.append((oh, ow, ents))
    if split_branches:
        assert len(out) == 27
    else:
        assert len(out) == 25
    assert sum(len(e) for _, _, e in out) == 81
    return out


def _build_program(with_bias):
    nc = bass.Bass()
    xin = nc.dram_tensor("x", [NP, NSLAB * HW], BF16, kind="ExternalInput")
    masks_in = nc.dram_tensor("masks", [NP, 7 * NP], BF16, kind="ExternalInput")
    cwx_in = nc.dram_tensor("cwx", [NP, NSLAB * 27 * NB], F32, kind="ExternalInput")
    cbx_in = nc.dram_tensor("cbx", [NP, NSLAB * NB], F32, kind="ExternalInput")
    gdin = nc.dram_tensor("gd", [G], F32, kind="ExternalInput")
    w1t_in = nc.dram_tensor("w1t", [HID, C + G], F32, kind="ExternalInput")
    b1_in = nc.dram_tensor("b1", [HID], F32, kind="ExternalInput")
    w2_in = nc.dram_tensor("w2", [HID, NB], F32, kind="ExternalInput")
    b2_in = nc.dram_tensor("b2", [NB], F32, kind="ExternalInput")
    lng_in = nc.dram_tensor("lng", [C + G], F32, kind="ExternalInput")
    lnb_in = nc.dram_tensor("lnb", [C + G], F32, kind="ExternalInput")
    dm_in = nc.dram_tensor("dm", [NP, 4], F32, kind="ExternalInput")
    yout = nc.dram_tensor("y", [NP, NSLAB * HW], F32, kind="ExternalOutput")

    p_ung = _pass_list(True)
    p_gat = _pass_list(False)

    with tile.TileContext(nc) as tc:
        with (
            tc.tile_pool(name="sbuf", bufs=1) as pool,
            tc.tile_pool(name="mats", bufs=2) as matpool,
            tc.tile_pool(name="outs", bufs=4) as outpool,
            tc.tile_pool(name="dram", bufs=1, space="DRAM") as dpool,
            tc.tile_pool(name="psum", bufs=1, space="PSUM") as ppool,
        ):
            xs = [
                pool.tile([NP, HW], BF16, tag=f"xs{s}", name=f"xs{s}")
                for s in range(NSLAB)
            ]
            masks = pool.tile([NP, 7 * NP], BF16, tag="masks")
            cwx = pool.tile([NP, NSLAB * 27 * NB], F32, tag="cwx")
            w_exp = pool.tile([NP, NSLAB * 27 * NB], F32, tag="w_exp")
            scr = pool.tile([NP, HW], BF16, tag="scr")
            part = pool.tile([NP, NSLAB], F32, tag="part")
            grow = pool.tile([1, 2 * NP * NSLAB], F32, tag="grow")
            xr_p1 = pool.tile([NP, HW], BF16, tag="xr_p1")
            xr_m1 = pool.tile([NP, HW], BF16, tag="xr_m1")
            xr_p2 = pool.tile([NP, HW], BF16, tag="xr_p2")
            xr_m2 = pool.tile([NP, HW], BF16, tag="xr_m2")
            acc5 = pool.tile([NP, HW], F32, tag="acc5")
            wz = pool.tile([NP, 16], F32, tag="wz")
            dmask = pool.tile([NP, 4], F32, tag="dmask")
            bb = [
                [
                    pool.tile([NP, HW], BF16, tag=f"bb{s}_{b}",
                              name=f"bb{s}_{b}")
                    for b in range(NB)
                ]
                for s in range(NUNG)
            ]
            g_row = pool.tile([1, C + G], F32, tag="g_row")
            gd_row = pool.tile([1, C + G], F32, tag="gd_row")
            lng = pool.tile([1, C + G], F32, tag="lng")
            lnb = pool.tile([1, C + G], F32, tag="lnb")
            gn_row = pool.tile([1, C + G], F32, tag="gn_row")
            gn_bc = pool.tile([HID, C + G], F32, tag="gn_bc")
            w1t = pool.tile([HID, C + G], F32, tag="w1t")
            prod = pool.tile([HID, C + G], F32, tag="prod")
            hvec = pool.tile([HID, 1], F32, tag="hvec")
            b1c = pool.tile([HID, 1], F32, tag="b1c")
            w2t = pool.tile([HID, NB], F32, tag="w2t")
            l2tmp = pool.tile([HID, NB], F32, tag="l2tmp")
            z72 = pool.tile([1, HID * NB], F32, tag="z72")
            zrow = pool.tile([1, NB], F32, tag="zrow")
            b2r = pool.tile([1, NB], F32, tag="b2r")
            wts = pool.tile([1, NB], F32, tag="wts")
            wts_bc = pool.tile([NP, NB], F32, tag="wts_bc")
            s1 = pool.tile([1, 1], F32, tag="s1")
            s2 = pool.tile([1, 1], F32, tag="s2")
            s3 = pool.tile([1, 1], F32, tag="s3")
            s4 = pool.tile([1, 1], F32, tag="s4")
            if with_bias:
                cbx = pool.tile([NP, NSLAB * NB], F32, tag="cbx")
                b_exp = pool.tile([NP, NSLAB], F32, tag="b_exp")
                betmp = pool.tile([NP, NSLAB * NB], F32, tag="betmp")

            cin = dpool.tile([NP, NSLAB], F32, tag="cin")
            cout = dpool.tile([2 * NP, NSLAB], F32, tag="cout")
            gb = dpool.tile([1, C + G], F32, tag="gb")
            wb = dpool.tile([1, NB], F32, tag="wb")

            v = nc.vector
            sc = nc.scalar

            # ---- A: loads (small weights first, then x slab-by-slab) ----
            nc.sync.dma_start(out=masks[:, :], in_=masks_in[:, :])
            nc.sync.dma_start(out=cwx[:, :], in_=cwx_in[:, :])
            nc.sync.dma_start(out=xs[0][:, :], in_=xin[:, 0:HW])
            for s in range(1, NSLAB):
                nc.sync.dma_start(
                    out=xs[s][:, :], in_=xin[:, s * HW : (s + 1) * HW]
                )
            nc.sync.dma_start(out=w1t[:, :], in_=w1t_in[:, :])
            nc.sync.dma_start(out=b1c[:, :], in_=b1_in[:, None])
            nc.sync.dma_start(out=w2t[:, :], in_=w2_in[:, :])
            nc.sync.dma_start(out=b2r[:, :], in_=b2_in[None, :])
            nc.sync.dma_start(out=lng[:, :], in_=lng_in[None, :])
            nc.sync.dma_start(out=lnb[:, :], in_=lnb_in[None, :])
            nc.sync.dma_start(out=g_row[:, C:], in_=gdin[None, :])
            nc.sync.dma_start(out=dmask[:, :], in_=dm_in[:, :])
            if with_bias:
                nc.sync.dma_start(out=cbx[:, :], in_=cbx_in[:, :])

            # ---- B: plane sums: slabs 0-3 on Act now; slab 4 threaded
            # between slab-0 branch copies; slab 5 on DVE (Act stays just
            # ahead of both the PSUM-copy demand and the collective) ----
            for s in range(NSLAB - 2):
                sc.activation(
                    scr[:, :], xs[s][:, :], ACTF.Copy,
                    accum_out=part[:, s : s + 1],
                )

            # ---- C: pairwise AllGather of raw plane sums ----
            nc.sync.dma_start(out=cin[:, :], in_=part[:, :])
            nc.gpsimd.collective_compute(
                "AllGather",
                ALU.bypass,
                replica_groups=[[2 * b, 2 * b + 1] for b in range(B)],
                ins=[cin.opt()],
                outs=[cout.opt()],
            )
            nc.sync.dma_start(out=grow[:, :], in_=cout[:, :])

            # ---- conv helpers ----
            def build_mats(s, passes, wsrc):
                mats = []
                for mi, (oh, ow, entries) in enumerate(passes):
                    mt = matpool.tile([NP, NP], BF16, tag=f"m{mi}")
                    for ei, (od, t) in enumerate(entries):
                        mk_in = masks[:, (od + 3) * NP : (od + 4) * NP]
                        wcol = wsrc[:, s * 81 + t : s * 81 + t + 1]
                        if ei == 0:
                            v.tensor_scalar(
                                out=mt[:, :], in0=mk_in, scalar1=wcol,
                                scalar2=None, op0=ALU.mult,
                            )
                        else:
                            v.scalar_tensor_tensor(
                                out=mt[:, :], in0=mk_in, scalar=wcol,
                                in1=mt[:, :], op0=ALU.mult, op1=ALU.add,
                            )
                    mats.append(mt)
                return mats

            def emit_pass(ps, pv, mt, xf, xv, ci, oh, ow, start, stop):
                if (oh, ow) == (0, 0):
                    nc.tensor.matmul(
                        ps[:, :], mt[:, :],
                        xf[:, ci * CHUNK : (ci + 1) * CHUNK],
                        start=start, stop=stop, skip_group_check=True,
                    )
                    return
                h0 = max(ci * ROWS, -oh if oh < 0 else 0)
                h1 = min(ci * ROWS + ROWS, H - (oh if oh > 0 else 0))
                if h1 <= h0:
                    return
                w0 = -ow if ow < 0 else 0
                w1 = W - (ow if ow > 0 else 0)
                nc.tensor.matmul(
                    pv[:, h0 - ci * ROWS : h1 - ci * ROWS, w0:w1],
                    mt[:, :],
                    xv[:, h0 + oh : h1 + oh, w0 + ow : w1 + ow],
                    start=start, stop=stop, skip_group_check=True,
                )

            def slab_views(s):
                xf = xs[s][:, :]
                return xf, xf.rearrange("p (h w) -> p h w", h=H, w=W)

            def emit_ungated_chunkmajor(s, mats):
                xf, xv = slab_views(s)
                for ci in range(N_CHUNKS):
                    for b in range(NB):
                        ps = ppool.tile([NP, CHUNK], F32,
                                        tag=f"ps{(3 * ci + b) % 8}",
                                        name=f"ups{s}_{b}_{ci}")
                        pv = ps[:, :].rearrange("p (h w) -> p h w", h=ROWS, w=W)
                        for k in range(9):
                            oh, ow, _ = p_ung[b * 9 + k]
                            emit_pass(ps, pv, mats[b * 9 + k], xf, xv, ci,
                                      oh, ow, k == 0, k == 8)
                        sc.activation(
                            bb[s][b][:, ci * CHUNK : (ci + 1) * CHUNK],
                            ps[:, :], ACTF.Copy,
                        )

            def emit_merge(s):
                for ci in range(N_CHUNKS):
                    sl = slice(ci * CHUNK, (ci + 1) * CHUNK)
                    ot = outpool.tile([NP, CHUNK], F32, tag=f"o{ci % 4}")
                    v.tensor_scalar(
                        out=ot[:, :], in0=bb[s][0][:, sl],
                        scalar1=wts_bc[:, 0:1], scalar2=None, op0=ALU.mult,
                    )
                    for b in (1, 2):
                        v.scalar_tensor_tensor(
                            out=ot[:, :], in0=bb[s][b][:, sl],
                            scalar=wts_bc[:, b : b + 1], in1=ot[:, :],
                            op0=ALU.mult, op1=ALU.add,
                        )
                    if with_bias:
                        v.tensor_scalar(
                            out=ot[:, :], in0=ot[:, :],
                            scalar1=b_exp[:, s : s + 1], scalar2=None,
                            op0=ALU.add,
                        )
                    nc.sync.dma_start(
                        out=yout[:, s * HW + ci * CHUNK : s * HW + (ci + 1) * CHUNK],
                        in_=ot[:, :],
                    )

            def emit_gated_out(s, ci, ps, acc=None):
                ot = outpool.tile([NP, CHUNK], F32, tag=f"o{ci % 4}",
                                  name=f"ot{s}_{ci}")
                sc.activation(ot[:, :], ps[:, :], ACTF.Copy)
                if acc is not None:
                    v.tensor_tensor(
                        out=ot[:, :], in0=ot[:, :],
                        in1=acc[:, ci * CHUNK : (ci + 1) * CHUNK], op=ALU.add,
                    )
                if with_bias:
                    v.tensor_scalar(
                        out=ot[:, :], in0=ot[:, :],
                        scalar1=b_exp[:, s : s + 1], scalar2=None,
                        op0=ALU.add,
                    )
                nc.sync.dma_start(
                    out=yout[:, s * HW + ci * CHUNK : s * HW + (ci + 1) * CHUNK],
                    in_=ot[:, :],
                )

            def emit_gated_slab(s, passes=None, acc=None):
                passes = passes or p_gat
                mats = build_mats(s, passes, w_exp)
                if acc is not None:
                    emit_assist()
                xf, xv = slab_views(s)
                for ci in range(N_CHUNKS):
                    ps = ppool.tile([NP, CHUNK], F32, tag=f"ps{ci}",
                                    name=f"gps{s}_{ci}")
                    pv = ps[:, :].rearrange("p (h w) -> p h w", h=ROWS, w=W)
                    for mi, (oh, ow, _) in enumerate(passes):
                        emit_pass(ps, pv, mats[mi], xf, xv, ci, oh, ow,
                                  mi == 0, mi == len(passes) - 1)
                    emit_gated_out(s, ci, ps, acc)

            # ---- D: slab 0 ungated (builds + matmuls + copies) ----
            def red4():
                sc.activation(
                    scr[:, :], xs[4][:, :], ACTF.Copy,
                    accum_out=part[:, 4:5],
                )

            # p-state warmup: dummy matmuls on the masks tile while the
            # xs0 DMA is still in flight, so the real conv stream starts at
            # full clock (the tensor engine needs ~3us of continuous work)
            wps = ppool.tile([NP, CHUNK], F32, tag="ps7", name="warmps")
            for wi in range(8):
                nc.tensor.matmul(
                    wps[:, :], masks[:, 3 * NP : 4 * NP],
                    masks[:, 2 * NP : 2 * NP + CHUNK],
                    start=(wi == 0), stop=(wi == 7), skip_group_check=True,
                )

            mats0 = build_mats(0, p_ung, cwx)
            emit_ungated_chunkmajor(0, mats0)
            red4()

            # slab-5 plane sum on DVE (Act is busy; DVE has a lull here)
            v.reduce_sum(
                part[:, NSLAB - 1 : NSLAB], xs[NSLAB - 1][:, :],
                axis=mybir.AxisListType.X,
            )

            # ---- F1: slab 1 ungated ----
            mats1 = build_mats(1, p_ung, cwx)
            emit_ungated_chunkmajor(1, mats1)

            # ---- E: gate MLP ----
            # feat[48r + 8s + c] = sum_d cout[r, (c,d), s] / (D*HW)
            for r in range(2):
                gview = grow[:, r * NP * NSLAB : (r + 1) * NP * NSLAB].rearrange(
                    "a (c d s) -> a s c d", c=NCH, d=D, s=NSLAB
                )
                tview = g_row[:, 48 * r : 48 * r + 48].rearrange(
                    "a (s c) -> a s c", s=NSLAB, c=NCH
                )
                v.reduce_sum(tview, gview, axis=mybir.AxisListType.X)
            v.tensor_scalar_mul(g_row[:, 0:C], g_row[:, 0:C], 1.0 / (D * HW))

            # LayerNorm over 192 on one partition
            v.reduce_sum(s1[:, :], g_row[:, :], axis=mybir.AxisListType.X)
            v.tensor_scalar_mul(s1[:, :], s1[:, :], 1.0 / (C + G))  # mu
            v.tensor_scalar(
                out=gd_row[:, :], in0=g_row[:, :], scalar1=s1[:, :], scalar2=None,
                op0=ALU.subtract,
            )
            v.tensor_tensor(out=gn_row[:, :], in0=gd_row[:, :], in1=gd_row[:, :], op=ALU.mult)
            v.reduce_sum(s2[:, :], gn_row[:, :], axis=mybir.AxisListType.X)
            v.tensor_scalar(
                out=s2[:, :], in0=s2[:, :], scalar1=1.0 / (C + G), scalar2=LN_EPS,
                op0=ALU.mult, op1=ALU.add,
            )  # var + eps
            sc.activation(s3[:, :], s2[:, :], ACTF.Sqrt)
            # one Newton step for a clean sqrt
            v.reciprocal(s4[:, :], s3[:, :])
            v.tensor_tensor(out=s4[:, :], in0=s4[:, :], in1=s2[:, :], op=ALU.mult)
            v.tensor_tensor(out=s4[:, :], in0=s4[:, :], in1=s3[:, :], op=ALU.add)
            v.tensor_scalar_mul(s4[:, :], s4[:, :], 0.5)
            v.reciprocal(s3[:, :], s4[:, :])  # rstd
            v.tensor_scalar(
                out=gn_row[:, :], in0=gd_row[:, :], scalar1=s3[:, :], scalar2=None,
                op0=ALU.mult,
            )
            v.tensor_tensor(out=gn_row[:, :], in0=gn_row[:, :], in1=lng[:, :], op=ALU.mult)
            v.tensor_tensor(out=gn_row[:, :], in0=gn_row[:, :], in1=lnb[:, :], op=ALU.add)

            # MLP layer 1: h = gelu(gn @ w1 + b1) via row-products
            nc.sync.dma_start(out=gb[:, :], in_=gn_row[:, :])
            nc.sync.dma_start(out=gn_bc[:, :], in_=gb[:1, :].partition_broadcast(HID))
            v.tensor_tensor(out=prod[:, :], in0=w1t[:, :], in1=gn_bc[:, :], op=ALU.mult)
            v.reduce_sum(hvec[:, :], prod[:, :], axis=mybir.AxisListType.X)
            v.tensor_tensor(out=hvec[:, :], in0=hvec[:, :], in1=b1c[:, :], op=ALU.add)
            sc.activation(hvec[:, :], hvec[:, :], ACTF.Gelu)

            # MLP layer 2 via DRAM transpose bounce
            v.tensor_scalar(
                out=l2tmp[:, :], in0=w2t[:, :], scalar1=hvec[:, :], scalar2=None,
                op0=ALU.mult,
            )
            nc.sync.dma_start(out=z72[:, :], in_=l2tmp[:, :])
            z3 = z72[:, :].rearrange("a (j i) -> a j i", j=HID, i=NB)
            for i in range(NB):
                v.reduce_sum(zrow[:, i : i + 1], z3[:, :, i], axis=mybir.AxisListType.X)
            v.tensor_tensor(out=zrow[:, :], in0=zrow[:, :], in1=b2r[:, :], op=ALU.add)

            # softmax over 3
            v.reduce_max(s1[:, :], zrow[:, :], axis=mybir.AxisListType.X)
            v.tensor_scalar(
                out=zrow[:, :], in0=zrow[:, :], scalar1=s1[:, :], scalar2=None,
                op0=ALU.subtract,
            )
            sc.activation(zrow[:, :], zrow[:, :], ACTF.Exp)
            v.reduce_sum(s2[:, :], zrow[:, :], axis=mybir.AxisListType.X)
            v.reciprocal(s2[:, :], s2[:, :])
            v.tensor_scalar(
                out=wts[:, :], in0=zrow[:, :], scalar1=s2[:, :], scalar2=None,
                op0=ALU.mult,
            )

            # broadcast gate weights; fold into per-channel tap weights
            nc.sync.dma_start(out=wb[:, :], in_=wts[:, :])
            nc.sync.dma_start(out=wts_bc[:, :], in_=wb[:1, :].partition_broadcast(NP))
            for s in range(NUNG, NSLAB):
                for i in range(NB):
                    sl = slice(s * 81 + i * 27, s * 81 + (i + 1) * 27)
                    v.tensor_scalar(
                        out=w_exp[:, sl], in0=cwx[:, sl],
                        scalar1=wts_bc[:, i : i + 1], scalar2=None, op0=ALU.mult,
                    )
            if with_bias:
                for i in range(NB):
                    v.tensor_scalar(
                        out=betmp[:, i::NB], in0=cbx[:, i::NB],
                        scalar1=wts_bc[:, i : i + 1], scalar2=None, op0=ALU.mult,
                    )
                v.tensor_tensor(
                    out=b_exp[:, :], in0=betmp[:, 0::NB], in1=betmp[:, 1::NB],
                    op=ALU.add,
                )
                v.tensor_tensor(
                    out=b_exp[:, :], in0=b_exp[:, :], in1=betmp[:, 2::NB],
                    op=ALU.add,
                )

            # slab 5: move 4 branch-1 passes off the PE onto the Vector
            # engine, which is idle for the last ~70us.  od=+-1 depth taps
            # read partition-rotated copies of the slab; rows whose rotated
            # partner crosses a channel block get weight 0 via dmask.
            ASSIST = [(0, 0, -1), (0, 0, 1), (0, -1, 0), (0, 1, 0),
                      (1, 0, -2)]
            nc.sync.dma_start(out=xr_p1[0 : NP - 1, :], in_=xs[5][1:NP, :])
            nc.sync.dma_start(out=xr_p1[NP - 1 : NP, :], in_=xs[5][NP - 1 : NP, :])
            nc.sync.dma_start(out=xr_m1[1:NP, :], in_=xs[5][0 : NP - 1, :])
            nc.sync.dma_start(out=xr_m1[0:1, :], in_=xs[5][0:1, :])
            nc.sync.dma_start(out=xr_p2[0 : NP - 2, :], in_=xs[5][2:NP, :])
            nc.sync.dma_start(out=xr_p2[NP - 2 : NP, :], in_=xs[5][NP - 2 : NP, :])
            nc.sync.dma_start(out=xr_m2[2:NP, :], in_=xs[5][0 : NP - 2, :])
            nc.sync.dma_start(out=xr_m2[0:2, :], in_=xs[5][0:2, :])
            nc.gpsimd.memset(acc5[:, :], 0.0)
            rot_src = {1: xr_p1, -1: xr_m1, 2: xr_p2, -2: xr_m2}
            rot_msk = {1: 0, -1: 1, 2: 2, -2: 3}

            def emit_assist():
                k = 0
                accv = acc5[:, :].rearrange("p (h w) -> p h w", h=H, w=W)
                for br, oh, ow in ASSIST:
                    d = DILS[br]
                    for kd in range(K):
                        od = (kd - 1) * d
                        t = (br * 27 + kd * 9 + (oh // d + 1) * 3
                             + (ow // d + 1))
                        wcol = w_exp[:, 5 * 81 + t : 5 * 81 + t + 1]
                        if od == 0:
                            src, wc = xs[5], wcol
                        else:
                            v.tensor_tensor(
                                out=wz[:, k : k + 1], in0=wcol,
                                in1=dmask[:, rot_msk[od] : rot_msk[od] + 1],
                                op=ALU.mult,
                            )
                            src = rot_src[od]
                            wc = wz[:, k : k + 1]
                            k += 1
                        sv = src[:, :].rearrange("p (h w) -> p h w", h=H, w=W)
                        h0, h1 = max(0, -oh), H - max(0, oh)
                        w0, w1 = max(0, -ow), W - max(0, ow)
                        v.scalar_tensor_tensor(
                            out=accv[:, h0:h1, w0:w1],
                            in0=sv[:, h0 + oh : h1 + oh, w0 + ow : w1 + ow],
                            scalar=wc, in1=accv[:, h0:h1, w0:w1],
                            op0=ALU.mult, op1=ALU.add,
                        )

            rm = {(oh, ow) for _, oh, ow in ASSIST}
            p_lite = [e for e in p_gat if (e[0], e[1]) not in rm]

            # ---- G..J: gated slabs; merges (not PE-critical) trail ----
            emit_gated_slab(2)
            emit_merge(0)
            emit_gated_slab(3)
            emit_merge(1)
            emit_gated_slab(4)
            emit_gated_slab(5, passes=p_lite, acc=acc5)

    _split_sem_waits(nc)
    return nc


_WAITSPLIT = [0]


def _split_sem_waits(nc, max_waits=1):
    """This walrus build rejects >1 SyncWait per instruction (and any wait on
    a Drain). Move excess waits onto same-engine NOPs inserted just before."""
    for bb in nc.main_func.blocks:
        insns = bb.instructions
        i = 0
        while i < len(insns):
            ins = insns[i]
            si = ins.sync_info
            limit = 0 if ins.opcode == "Drain" else max_waits
            if si is not None and si.on_wait is not None and len(si.on_wait) > limit:
                waits = list(si.on_wait)
                keep = waits[-limit:] if limit else []
                extra = waits[: len(waits) - limit]
                pos = i
                for j in range(0, len(extra), max_waits):
                    nop = mybir.InstNoOp(
                        name=f"I-waitsplit-{_WAITSPLIT[0]}", ins=[], outs=[]
                    )
                    _WAITSPLIT[0] += 1
                    nop.engine = ins.engine
                    nop.sync_info = mybir.SyncInfo(
                        on_wait=extra[j : j + max_waits], on_update=[]
                    )
                    insns.insert(pos, nop)
                    pos += 1
                    i += 1
                si.on_wait = keep
            i += 1


def _make_masks():
    m = np.zeros((NP, 7 * NP), dtype=np.float32)
    for od in range(-3, 4):
        for p in range(NP):
            q = p - od
            if q // D == p // D and 0 <= q < NP:
                m[p, (od + 3) * NP + q] = 1.0
    return m.astype(ml_dtypes.bfloat16)


def _prep_inputs(x, guidance, convw, convb, ln_g, ln_b, w1, b1, w2, b2):
    f = np.float32
    w3 = np.ascontiguousarray(convw.reshape(NB, C, 27), dtype=f)
    cb = np.ascontiguousarray(convb, dtype=f)
    dm = np.ones((NP, 4), dtype=np.float32)
    dm[D - 1 :: D, 0] = 0.0   # od=+1: d==15 rows read the next c-block
    dm[0::D, 1] = 0.0         # od=-1: d==0 rows read the previous c-block
    dm[D - 2 :: D, 2] = 0.0   # od=+2: d in {14,15}
    dm[D - 1 :: D, 2] = 0.0
    dm[0::D, 3] = 0.0         # od=-2: d in {0,1}
    dm[1::D, 3] = 0.0
    common = dict(
        masks=_make_masks(),
        dm=dm,
        w1t=np.ascontiguousarray(w1.T, dtype=f),
        b1=np.ascontiguousarray(b1, dtype=f),
        w2=np.ascontiguousarray(w2, dtype=f),
        b2=np.ascontiguousarray(b2, dtype=f),
        lng=np.ascontiguousarray(ln_g, dtype=f),
        lnb=np.ascontiguousarray(ln_b, dtype=f),
    )
    in_maps = []
    for core in range(N_CORES):
        b, hc = core // 2, core % 2
        ch0 = 48 * hc
        # xs[p=c*16+d, s*HW+j] = x[b, ch0+8s+c, d, j]
        arr = np.ascontiguousarray(x[b, ch0 : ch0 + 48], dtype=f)
        arr = arr.reshape(NSLAB, NCH, D, HW).transpose(1, 2, 0, 3).reshape(
            NP, NSLAB * HW
        )
        # cwx[p=c*16+d, s*81+t] = convw[br, ch0+8s+c, t27]  (d-independent)
        cw = w3[:, ch0 : ch0 + 48, :].reshape(NB, NSLAB, NCH, 27)
        cw = cw.transpose(2, 1, 0, 3).reshape(NCH, NSLAB * NB * 27)
        cwx = np.repeat(cw, D, axis=0)  # row c*16+d <- cw[c]
        cbs = cb[:, ch0 : ch0 + 48].reshape(NB, NSLAB, NCH)
        cbs = cbs.transpose(2, 1, 0).reshape(NCH, NSLAB * NB)
        cbx = np.repeat(cbs, D, axis=0)
        in_maps.append(
            dict(
                x=arr.astype(ml_dtypes.bfloat16),
                gd=np.ascontiguousarray(guidance[b], dtype=f),
                cwx=np.ascontiguousarray(cwx, dtype=f),
                cbx=np.ascontiguousarray(cbx, dtype=f),
                **common,
            )
        )
    return in_maps


_CACHED_NC = {}


def kernel(x, guidance, convw, convb, ln_g, ln_b, w1, b1, w2, b2):
    with_bias = bool(np.any(np.asarray(convb)))
    if with_bias not in _CACHED_NC:
        _CACHED_NC[with_bias] = _build_program(with_bias)
    nc = _CACHED_NC[with_bias]
    globals()["_LAST_NC"] = nc
    in_maps = _prep_inputs(
        x, guidance, convw, convb, ln_g, ln_b, w1, b1, w2, b2
    )
    res = run_bass_kernel_spmd(nc, in_maps, list(range(N_CORES)))
    out = np.empty((B, C, D, H, W), dtype=np.float32)
    for core in range(N_CORES):
        b, hc = core // 2, core % 2
        y = res.results[core]["y"].reshape(NCH, D, NSLAB, HW)
        out[b, 48 * hc : 48 * hc + 48] = (
            y.transpose(2, 0, 1, 3).reshape(48, D, H, W)
        )
    return out


if __name__ == "__main__":
    rng = np.random.default_rng(0)
    ins = dict(
        x=rng.standard_normal((B, C, D, H, W), dtype=np.float32),
        guidance=rng.standard_normal((B, G), dtype=np.float32),
        convw=(rng.standard_normal((NB, C, 1, K, K, K)) * 0.1).astype(np.float32),
        convb=np.zeros((NB, C), np.float32),
        ln_g=np.ones((C + G,), np.float32),
        ln_b=np.zeros((C + G,), np.float32),
        w1=(rng.standard_normal((C + G, HID)) * 0.05).astype(np.float32),
        b1=np.zeros((HID,), np.float32),
        w2=(rng.standard_normal((HID, NB)) * 0.05).astype(np.float32),
        b2=np.zeros((NB,), np.float32),
    )
    out = kernel(**ins)
    print("kernel ran, out shape", out.shape, "mean", float(np.abs(out).mean()))
